# revision 9
# baseline (speedup 1.0000x reference)
"""Trainium2 Bass kernel for nn_AlignedGloveLayer (retrieval_knn).

TimelineSim per-core estimate 128.6us (prior baseline 207.5us); hardware
rel err vs the fp32 jax reference ~1.3e-4.

Sharding (8 NeuronCores, SPMD): each core runs the small MLPs for its own
1024 queries, holds all 8192 check rows as fp8 stationaries, and emits
per-check-row statistics over its query range; the host min/softmin-combines
the 8 shards.

Per-core engine plan:
  - check-row norms (bb) are host-side input preprocessing.
  - The 128 cdist tiles' [128, 1024] psum reductions split across two lanes:
    DVE tensor_reduce(min) for NV1+NV2 tiles, ACT Exp softmin (in-place on
    psum, accum_out) for the rest; device pivots for both cdists.
  - +aa[i]/+gg[i] via one fp8 DoubleRow K=2 matmul per psum half (aa split
    hi+lo fp8 rows for precision).
  - Cycle-consistency reuses bf16 copies A_bf/G_bf of the MLP outputs; the
    (W2 h + b - x) difference is accumulated in psum via -identity and
    bias-outer-product matmuls, squared on ACT.
  - All small constants ride in two packed blob DMAs (HWDGE is serial at
    ~625ns/DMA); yc/xc stationaries stream in 4 chunks each.
"""

import numpy as np
import ml_dtypes

BF = ml_dtypes.bfloat16
F32 = np.float32
F8 = ml_dtypes.float8_e4m3

B = 8192          # query batch
S = B // 8        # per-core query shard
DX, DY, H = 512, 256, 100
P = 128
GX, GY = DX // P, DY // P   # 4, 2 contraction groups
MX, MY = DX // P, DY // P

BETA1, POFF1 = 25.0, 2.5
BETA2, POFF2 = 20.0, 3.0

# lane maps: True -> DVE tensor_reduce (exact min), False -> ACT softmin.
# cdist1: evens + the early odd tiles (ACT is busy with the MLP prologue
# when jt 1..7 drain); cdist2: odds. Strict v,a alternation mid-kernel.
XTRA1 = (1, 3, 5)     # early odds -> DVE (ACT busy with prologue)
FLIP1 = ()                # evens -> ACT (balance knob)

# bf16 const blob column offsets
CW_FX1 = 0            # [128, 4*100]
CW_GY1 = 400          # [128, 2*100]
CW_FX2 = 600          # [100, 256]
CW_GY2 = 856          # [100, 512]
C_ONES = 1368         # [128, 128]
C_NEGI = 1496         # [128, 128]
C_ONESR = 1624        # [1, 512]
C_FXB2T = 2136        # [1, 256]
C_GYB2T = 2392        # [1, 512]
NBLOB = 2904


LANE1 = [((jt % 2 == 0) or (jt in XTRA1)) and (jt not in FLIP1)
         for jt in range(64)]
LANE2 = [(jt % 2 == 1) for jt in range(64)]

TRACE = False
_CACHE = {}


def _legalize_sync(nc, max_total=2, max_ev_waits=2):
    """This container's walrus build rejects instructions carrying more than
    one sync wait (and ~2 sync commands total). Tile attaches full
    vector-clock waits to instructions, so split excess waits onto preceding
    same-engine InstEventSemaphore instructions — engine streams execute in
    order, so a wait executed earlier on the same engine preserves every
    happens-before edge."""
    import concourse.mybir as mybir

    n_new = 0
    for f in nc.m.functions:
        for blk in f.blocks:
            insts = blk.instructions
            need = False
            for inst in insts:
                si = inst.sync_info
                if si is not None and len(si.on_wait) > max(
                        0, min(1, max_total - len(si.on_update))):
                    need = True
                    break
            if not need:
                continue
            out = []
            for inst in insts:
                si = inst.sync_info
                if si is not None:
                    waits = list(si.on_wait)
                    ups = list(si.on_update)
                    assert len(ups) <= max_total, (
                        f"{inst.name}: {len(ups)} sync updates, cannot legalize")
                    keep_w = max(0, min(1, max_total - len(ups)))
                    if len(waits) > keep_w:
                        spill = waits[:len(waits) - keep_w]
                        kept = waits[len(waits) - keep_w:]
                        for k in range(0, len(spill), max_ev_waits):
                            ev = mybir.InstEventSemaphore(
                                name=f"legalw-{nc.next_id()}",
                                engine=inst.engine,
                                ins=[], outs=[],
                                sync_info=mybir.SyncInfo(
                                    on_wait=spill[k:k + max_ev_waits],
                                    on_update=[]),
                            )
                            nc.register_instruction(ev)
                            out.append(ev)
                            n_new += 1
                        inst.sync_info = mybir.SyncInfo(
                            on_wait=kept, on_update=ups)
                out.append(inst)
            blk.instructions = out
    return n_new


def _build_nc():
    import concourse.bass as bass
    import concourse.mybir as mybir
    from concourse.tile import TileContext

    f32 = mybir.dt.float32
    bf16 = mybir.dt.bfloat16
    fp8 = mybir.dt.float8e4
    AF = mybir.ActivationFunctionType
    OP = mybir.AluOpType
    AX = mybir.AxisListType
    DR = mybir.MatmulPerfMode.DoubleRow

    nc = bass.Bass()
    ts = bass.ts

    # ---- DRAM I/O ----
    blobB = nc.dram_tensor("blobB", [P, NBLOB], bf16, kind="ExternalInput")
    blobF = nc.dram_tensor("blobF", [P, 8], f32, kind="ExternalInput")
    ones2in = nc.dram_tensor("ones2in", [1, 2, P], fp8, kind="ExternalInput")
    xpTsb = nc.dram_tensor("xpTsb", [DX, S], bf16, kind="ExternalInput")
    ypTsb = nc.dram_tensor("ypTsb", [DY, S], bf16, kind="ExternalInput")
    ycT2 = nc.dram_tensor("ycT2", [DY, B], fp8, kind="ExternalInput")  # -2*Yc^T
    xcT2 = nc.dram_tensor("xcT2", [DX, B], fp8, kind="ExternalInput")  # -2*Xc^T

    o_min1 = nc.dram_tensor("o_min1", [P, 64], f32, kind="ExternalOutput")
    o_min2 = nc.dram_tensor("o_min2", [P, 64], f32, kind="ExternalOutput")
    o_cfx = nc.dram_tensor("o_cfx", [1, S], f32, kind="ExternalOutput")
    o_cgy = nc.dram_tensor("o_cgy", [1, S], f32, kind="ExternalOutput")
    o_c1 = nc.dram_tensor("o_c1", [1, 1], f32, kind="ExternalOutput")
    o_c2 = nc.dram_tensor("o_c2", [1, 1], f32, kind="ExternalOutput")
    c1_dram = nc.dram_tensor("c1_dram", [1, 1], f32)
    c2_dram = nc.dram_tensor("c2_dram", [1, 1], f32)

    xpTsb_v = xpTsb[:].rearrange("(g p) n -> p g n", p=P)
    ypTsb_v = ypTsb[:].rearrange("(g p) n -> p g n", p=P)
    ycT2_v = ycT2[:].rearrange("(g p) n -> p g n", p=P)
    xcT2_v = xcT2[:].rearrange("(g p) n -> p g n", p=P)

    with TileContext(nc) as tc:
        with (
            tc.tile_pool(name="cpool", bufs=1) as cpool,
        ):
            # ---- ACT warmup: wait-free instructions for table loads ----
            warm = cpool.tile([1, 2], bf16, name="warm")
            nc.vector.memset(warm, 0.0)
            nc.scalar.activation(warm, warm, AF.Exp)
            nc.scalar.copy(warm, warm)
            nc.scalar.activation(warm, warm, AF.Relu)
            nc.scalar.activation(warm, warm, AF.Identity)
            nc.scalar.activation(warm, warm, AF.Square)

            # ---- inputs (HWDGE is serial: order = priority) ----
            blb = cpool.tile([P, NBLOB], bf16, name="blb")
            t_xpsb = cpool.tile([P, GX, S], bf16, name="t_xpsb")
            nc.sync.dma_start(out=t_xpsb[:, :, 0:512],
                              in_=xpTsb_v[:, :, 0:512])
            nc.sync.dma_start(out=blb[:, 0:CW_FX2],
                              in_=blobB[:, 0:CW_FX2])
            blf = cpool.tile([P, 8], f32, name="blf")
            nc.sync.dma_start(out=blf, in_=blobF[:])
            nc.sync.dma_start(out=blb[:, CW_FX2:C_ONES + P],
                              in_=blobB[:, CW_FX2:C_ONES + P])
            nc.sync.dma_start(out=t_xpsb[:, :, 512:1024],
                              in_=xpTsb_v[:, :, 512:1024])
            t_ypsb = cpool.tile([P, GY, S], bf16, name="t_ypsb")
            nc.sync.dma_start(out=t_ypsb, in_=ypTsb_v)
            nc.sync.dma_start(out=blb[:, C_ONES + P:NBLOB],
                              in_=blobB[:, C_ONES + P:NBLOB])
            ones2 = cpool.tile([1, 2, P], fp8, name="ones2")
            nc.sync.dma_start(out=ones2, in_=ones2in[:])
            t_yc = cpool.tile([P, GY, B], fp8, name="t_yc")
            t_xc = cpool.tile([P, GX, B], fp8, name="t_xc")
            for ch in range(4):
                chs = ts(ch, B // 4)
                nc.sync.dma_start(out=t_yc[:, :, chs], in_=ycT2_v[:, :, chs])
            for ch in range(4):
                chs = ts(ch, B // 4)
                nc.sync.dma_start(out=t_xc[:, :, chs], in_=xcT2_v[:, :, chs])

            # blob views
            def w_fx1(g):
                return blb[:, CW_FX1 + g * H:CW_FX1 + (g + 1) * H]

            def w_gy1(g):
                return blb[:, CW_GY1 + g * H:CW_GY1 + (g + 1) * H]

            w_fx2 = blb[0:H, CW_FX2:CW_FX2 + DY]
            w_gy2 = blb[0:H, CW_GY2:CW_GY2 + DX]
            onescol = blb[:, C_ONES:C_ONES + 1]
            negI = blb[:, C_NEGI:C_NEGI + P]
            onesr = blb[0:1, C_ONESR:C_ONESR + 512]
            fxb2T = blb[0:1, C_FXB2T:C_FXB2T + DY]
            gyb2T = blb[0:1, C_GYB2T:C_GYB2T + DX]
            b_fx1 = blf[0:H, 0:1]
            b_gy1 = blf[0:H, 1:2]

            def b_fx2(mg):
                return blf[:, 2 + mg:3 + mg]

            def b_gy2(mg):
                return blf[:, 4 + mg:5 + mg]

            aarow = cpool.tile([1, S], f32, name="aarow")
            ggrow = cpool.tile([1, S], f32, name="ggrow")
            aahl = cpool.tile([1, 2, S], fp8, name="aahl")
            gghl = cpool.tile([1, 2, S], fp8, name="gghl")
            A_loc = cpool.tile([P, MY, S], fp8, name="A_loc")
            A_bf = cpool.tile([P, MY, S], bf16, name="A_bf")
            G_loc = cpool.tile([P, MX, S], fp8, name="G_loc")
            G_bf = cpool.tile([P, MX, S], bf16, name="G_bf")
            o1_sb = cpool.tile([P, 64], f32, name="o1_sb")
            o2_sb = cpool.tile([P, 64], f32, name="o2_sb")
            bias1 = cpool.tile([P, 1], f32, name="bias1")
            bias2 = cpool.tile([P, 1], f32, name="bias2")
            sqA = cpool.tile([P, MY, S], bf16, name="sqA")
            sqG = cpool.tile([P, MX, S], bf16, name="sqG")
            dsqX = cpool.tile([P, MX, S], bf16, name="dsqX")
            dsqY = cpool.tile([P, MY, S], bf16, name="dsqY")
            h4P = cpool.tile([H, 4, 512], bf16, name="h4P")

            with (
                tc.tile_pool(name="spool", bufs=2) as spool,
            ):
                psp = tc.alloc_tile_pool(name="psp", bufs=4, space="PSUM")

                def emit_fx_mlp(nst, nsub=1):
                    # nsub=2 splits into 256-col substeps: shorter first-A
                    # latency for the aa critical path in the prologue
                    w = 512 // nsub
                    for hh in range(nsub):
                        sl = bass.ds(nst * 512 + hh * w, w)
                        ps_h = psp.tile([H, w], f32, name="ps_h", tag="sm",
                                        bufs=2)
                        for g in range(GX):
                            nc.tensor.matmul(ps_h, w_fx1(g),
                                             t_xpsb[:, g, sl],
                                             start=(g == 0),
                                             stop=(g == GX - 1))
                        h_sb = spool.tile([H, w], bf16, name="h_sb",
                                          tag="h_sb")
                        nc.scalar.activation(h_sb, ps_h, AF.Relu, bias=b_fx1)
                        for mg in range(MY):
                            ps_a = psp.tile([P, w], f32, name="ps_a",
                                            tag="sm", bufs=2)
                            nc.tensor.matmul(ps_a, w_fx2[:, ts(mg, P)],
                                             h_sb, start=True, stop=True)
                            nc.scalar.activation(A_loc[:, mg, sl], ps_a,
                                                 AF.Identity, bias=b_fx2(mg))
                            nc.vector.tensor_scalar(A_bf[:, mg, sl], ps_a,
                                                    b_fx2(mg), None, OP.add)
                            nc.vector.tensor_tensor(sqA[:, mg, sl],
                                                    A_bf[:, mg, sl],
                                                    A_bf[:, mg, sl],
                                                    OP.mult)

                def emit_fx_aa(nst):
                    sl = ts(nst, 512)
                    ps_aa = psp.tile([1, 512], f32, name="ps_aa", tag="mix",
                                     bufs=3)
                    for mg in range(MY):
                        nc.tensor.matmul(ps_aa, onescol, sqA[:, mg, sl],
                                         start=(mg == 0), stop=(mg == MY - 1))
                    nc.scalar.copy(aarow[0:1, sl], ps_aa)
                    nc.gpsimd.tensor_copy(aahl[0:1, 0, sl], aarow[0:1, sl])
                    nc.gpsimd.tensor_tensor(aahl[0:1, 1, sl], aarow[0:1, sl],
                                            aahl[0:1, 0, sl], OP.subtract)

                def emit_gy_mlp(nst):
                    sl = ts(nst, 512)
                    ps_h2 = psp.tile([H, 512], f32, name="ps_h2", tag="sm",
                                     bufs=2)
                    for g in range(GY):
                        nc.tensor.matmul(ps_h2, w_gy1(g), t_ypsb[:, g, sl],
                                         start=(g == 0), stop=(g == GY - 1))
                    h2_sb = spool.tile([H, 512], bf16, name="h2_sb",
                                       tag="h_sb")
                    nc.scalar.activation(h2_sb, ps_h2, AF.Relu, bias=b_gy1)
                    for mg in range(MX):
                        ps_g = psp.tile([P, 512], f32, name="ps_g",
                                        tag="sm", bufs=2)
                        nc.tensor.matmul(ps_g, w_gy2[:, ts(mg, P)],
                                         h2_sb, start=True, stop=True)
                        nc.scalar.activation(G_loc[:, mg, sl], ps_g,
                                             AF.Identity, bias=b_gy2(mg))
                        nc.vector.tensor_scalar(G_bf[:, mg, sl], ps_g,
                                                b_gy2(mg), None, OP.add)
                        nc.vector.tensor_tensor(sqG[:, mg, sl],
                                                G_bf[:, mg, sl],
                                                G_bf[:, mg, sl], OP.mult)

                def emit_gy_gg(nst):
                    sl = ts(nst, 512)
                    ps_gg = psp.tile([1, 512], f32, name="ps_gg", tag="mix",
                                     bufs=3)
                    for mg in range(MX):
                        nc.tensor.matmul(ps_gg, onescol, sqG[:, mg, sl],
                                         start=(mg == 0), stop=(mg == MX - 1))
                    nc.scalar.copy(ggrow[0:1, sl], ps_gg)
                    nc.gpsimd.tensor_copy(gghl[0:1, 0, sl], ggrow[0:1, sl])
                    nc.gpsimd.tensor_tensor(gghl[0:1, 1, sl], ggrow[0:1, sl],
                                            gghl[0:1, 0, sl], OP.subtract)

                def post_row(row, hl, c_dram, o_c, biasT, beta, poff):
                    # pivot = min(row) - poff -> biasT broadcast
                    c_sb = spool.tile([1, 1], f32, name="c_sb", tag="c_sb")
                    nc.vector.tensor_reduce(c_sb, row, axis=AX.X, op=OP.min)
                    nc.sync.dma_start(out=c_dram[:], in_=c_sb)
                    nc.sync.dma_start(out=o_c[:], in_=c_sb)
                    nc.gpsimd.dma_start(
                        out=biasT,
                        in_=bass.AP(tensor=c_dram, offset=0,
                                    ap=[[0, P], [1, 1]]))
                    nc.vector.tensor_scalar(biasT, biasT, beta, -beta * poff,
                                            OP.mult, OP.add)

                def emit_cd_tile(which, jt):
                    t_st, m_sb, hl, npair, o_sb, biasT, beta, lane = (
                        (t_yc, A_loc, aahl, 1, o1_sb, bias1, BETA1,
                         LANE1[jt]) if which == 0 else
                        (t_xc, G_loc, gghl, 2, o2_sb, bias2, BETA2,
                         LANE2[jt]))
                    jsl = ts(jt, P)
                    ps = psp.tile([P, 1024], f32, name="ps_cd", tag="mix",
                                  bufs=3)
                    for h in range(2):
                        isl = ts(h, 512)
                        ph = ps[:, ts(h, 512)]
                        for pr in range(npair):
                            nc.tensor.matmul(
                                ph, t_st[:, 2 * pr:2 * pr + 2, jsl],
                                m_sb[:, 2 * pr:2 * pr + 2, isl],
                                start=(pr == 0), stop=False, perf_mode=DR)
                        nc.tensor.matmul(ph, ones2, hl[:, :, isl],
                                         start=False, stop=True, perf_mode=DR)
                    if lane:
                        nc.vector.tensor_reduce(o_sb[:, jt:jt + 1], ps,
                                                axis=AX.X, op=OP.min)
                    else:
                        nc.scalar.activation(ps, ps, AF.Exp, bias=biasT,
                                             scale=-beta,
                                             accum_out=o_sb[:, jt:jt + 1])

                def emit_cycle_head(side, nst):
                    # side 0: x->y->x second stage gy(A_bf); side 1: fx(G_bf)
                    csl = ts(nst, 512)
                    w1, src_bf, b1h, ng = (
                        (w_gy1, A_bf, b_gy1, GY) if side == 0 else
                        (w_fx1, G_bf, b_fx1, GX))
                    ps_h4 = psp.tile([H, 512], f32, name="ps_h4",
                                     tag="sm", bufs=2)
                    for g in range(ng):
                        nc.tensor.matmul(ps_h4, w1(g), src_bf[:, g, csl],
                                         start=(g == 0), stop=(g == ng - 1))
                    nc.scalar.activation(h4P[:, 2 * side + nst, :], ps_h4,
                                         AF.Relu, bias=b1h)

                def emit_cycle_mgs(side, nst, mgl):
                    csl = ts(nst, 512)
                    w2, bT, xin, dsq_t = (
                        (w_gy2, gyb2T, t_xpsb, dsqX) if side == 0 else
                        (w_fx2, fxb2T, t_ypsb, dsqY))
                    h4_sb = h4P[:, 2 * side + nst, :]
                    for mg in mgl:
                        # psum accumulates W2 h + b - x; ACT squares it
                        ps_xr = psp.tile([P, 512], f32, name="ps_xr",
                                         tag="sm", bufs=2)
                        nc.tensor.matmul(ps_xr, w2[:, ts(mg, P)], h4_sb,
                                         start=True, stop=False)
                        nc.tensor.matmul(ps_xr, negI, xin[:, mg, csl],
                                         start=False, stop=False)
                        nc.tensor.matmul(ps_xr, bT[0:1, ts(mg, P)], onesr,
                                         start=False, stop=True)
                        nc.scalar.activation(dsq_t[:, mg, csl], ps_xr,
                                             AF.Square)

                def emit_cycle_sum(side, nst):
                    csl = ts(nst, 512)
                    nmg, dsq_t, o_c = ((MX, dsqX, o_cfx) if side == 0
                                       else (MY, dsqY, o_cgy))
                    ps_nfx = psp.tile([1, 512], f32, name="ps_nfx",
                                      tag="mix", bufs=3)
                    for mg in range(nmg):
                        nc.tensor.matmul(ps_nfx, onescol, dsq_t[:, mg, csl],
                                         start=(mg == 0), stop=(mg == nmg - 1))
                    st_fx = spool.tile([1, 512], f32, name="st_fx",
                                       tag="stage")
                    nc.scalar.copy(st_fx, ps_nfx)
                    nc.sync.dma_start(out=o_c[0:1, csl], in_=st_fx)

                # ---- schedule ----
                wmm = spool.tile([P, 512], bf16, name="wmm", bufs=1)
                nc.vector.memset(wmm, 0.0)
                for _ in range(2):
                    wps = psp.tile([P, 512], f32, name="wps", tag="sm",
                                   bufs=2)
                    nc.tensor.matmul(wps, wmm[:, 0:P], wmm,
                                     start=True, stop=True)
                emit_fx_mlp(0)
                emit_fx_mlp(1)
                emit_gy_mlp(0)
                emit_fx_aa(0)
                emit_fx_aa(1)
                post_row(aarow, aahl, c1_dram, o_c1, bias1, BETA1, POFF1)
                emit_gy_mlp(1)
                emit_gy_gg(0)
                emit_gy_gg(1)
                post_row(ggrow, gghl, c2_dram, o_c2, bias2, BETA2, POFF2)
                for jt in range(0, 4):
                    emit_cd_tile(0, jt)
                emit_cycle_head(0, 0)
                emit_cd_tile(0, 4)
                emit_cycle_head(0, 1)
                emit_cd_tile(0, 5)
                emit_cycle_head(1, 0)
                emit_cd_tile(0, 6)
                emit_cycle_head(1, 1)
                emit_cd_tile(0, 7)
                # (side, nst, mg chunk) trickled through the main loop
                cjobs = {9: (0, 0, (0, 1)), 11: (0, 0, (2, 3)),
                         15: (0, 1, (0, 1)), 17: (0, 1, (2, 3)),
                         21: (1, 0, (0, 1)), 23: (1, 1, (0, 1))}
                csums = {13: (0, 0), 19: (0, 1), 25: (1, 0), 29: (1, 1)}
                for jt in range(8, 64):
                    # DVE-lane member of the pair first
                    if LANE1[jt] or not LANE2[jt - 8]:
                        emit_cd_tile(0, jt)
                        emit_cd_tile(1, jt - 8)
                    else:
                        emit_cd_tile(1, jt - 8)
                        emit_cd_tile(0, jt)
                    if jt in cjobs:
                        emit_cycle_mgs(*cjobs[jt])
                    if jt in csums:
                        emit_cycle_sum(*csums[jt])
                    if jt == 55:
                        nc.sync.dma_start(out=o_min2[:, 0:48],
                                          in_=o2_sb[:, 0:48])
                nc.sync.dma_start(out=o_min1[:], in_=o1_sb)
                for jt in range(56, 64):
                    emit_cd_tile(1, jt)
                psp.release()
                nc.sync.dma_start(out=o_min2[:, 48:64], in_=o2_sb[:, 48:64])

    _legalize_sync(nc)
    nc.finalize()
    return nc


def _host_prep(inputs):
    """Gather/transpose/cast on host -> per-core input maps + bb norms."""
    xw = np.asarray(inputs['x_weight'], dtype=np.float32)
    yw = np.asarray(inputs['y_weight'], dtype=np.float32)
    xp = np.asarray(inputs['x_present']).astype(np.int64)
    yc = np.asarray(inputs['y_check']).astype(np.int64)
    yp = np.asarray(inputs['y_present']).astype(np.int64)
    xc = np.asarray(inputs['x_check']).astype(np.int64)

    def c(a, dt):
        return np.ascontiguousarray(a, dtype=dt)

    # packed bf16 const blob
    blobB = np.zeros((P, NBLOB), dtype=BF)
    fxW1 = np.asarray(inputs['fx_W1'], dtype=np.float32)  # [DX, H]
    gyW1 = np.asarray(inputs['gy_W1'], dtype=np.float32)  # [DY, H]
    for g in range(GX):
        blobB[:, CW_FX1 + g * H:CW_FX1 + (g + 1) * H] = \
            fxW1[g * P:(g + 1) * P, :].astype(BF)
    for g in range(GY):
        blobB[:, CW_GY1 + g * H:CW_GY1 + (g + 1) * H] = \
            gyW1[g * P:(g + 1) * P, :].astype(BF)
    blobB[0:H, CW_FX2:CW_FX2 + DY] = np.asarray(inputs['fx_W2']).astype(BF)
    blobB[0:H, CW_GY2:CW_GY2 + DX] = np.asarray(inputs['gy_W2']).astype(BF)
    blobB[:, C_ONES:C_ONES + P] = np.ones((P, P), dtype=BF)
    blobB[:, C_NEGI:C_NEGI + P] = (-np.eye(P)).astype(BF)
    blobB[0:1, C_ONESR:C_ONESR + 512] = np.ones((1, 512), dtype=BF)
    blobB[0:1, C_FXB2T:C_FXB2T + DY] = \
        np.asarray(inputs['fx_b2']).reshape(1, -1).astype(BF)
    blobB[0:1, C_GYB2T:C_GYB2T + DX] = \
        np.asarray(inputs['gy_b2']).reshape(1, -1).astype(BF)

    blobF = np.zeros((P, 8), dtype=F32)
    blobF[0:H, 0] = np.asarray(inputs['fx_b1']).reshape(-1)
    blobF[0:H, 1] = np.asarray(inputs['gy_b1']).reshape(-1)
    blobF[:, 2:4] = np.asarray(inputs['fx_b2']).reshape(MY, P).T
    blobF[:, 4:8] = np.asarray(inputs['gy_b2']).reshape(MX, P).T

    ycr = yw[yc]
    xcr = xw[xc]
    shared = {
        'blobB': blobB,
        'blobF': blobF,
        'ones2in': np.ones((1, 2, P), dtype=F8),
        'ycT2': c(-2.0 * ycr.T, F8),
        'xcT2': c(-2.0 * xcr.T, F8),
    }
    in_maps = []
    for cix in range(8):
        sl = slice(cix * S, (cix + 1) * S)
        m = dict(shared)
        m['xpTsb'] = c(xw[xp[sl]].T, BF)
        m['ypTsb'] = c(yw[yp[sl]].T, BF)
        in_maps.append(m)
    bb1 = np.sum(ycr.astype(np.float64) ** 2, axis=1)
    bb2 = np.sum(xcr.astype(np.float64) ** 2, axis=1)
    return in_maps, bb1, bb2


def _combine_side(results, key, ckey, lanes, beta, poff, bb):
    """Column-combine one cdist: exact-min cols by min over shards, softmin
    cols by pivot-rescaled sumexp; then +bb, clamp, sqrt, sum."""
    pivots = [float(r[ckey][0, 0]) - poff for r in results]
    cstar = min(pivots)
    stot = np.zeros((P, 64), np.float64)
    for r, pv in zip(results, pivots):
        stot += r[key].astype(np.float64) * np.exp(beta * (cstar - pv))
    soft = cstar - np.log(np.maximum(stot, 1e-300)) / beta
    hard = np.min(np.stack([r[key] for r in results]),
                  axis=0).astype(np.float64)
    comb = np.where(np.asarray(lanes)[None, :], hard, soft)
    d = comb.T.reshape(-1) + bb
    return np.sqrt(np.maximum(d, 0.0)).sum()


def _host_combine(results, bb1, bb2):
    tot = _combine_side(results, 'o_min1', 'o_c1', LANE1, BETA1, POFF1, bb1)
    tot += _combine_side(results, 'o_min2', 'o_c2', LANE2, BETA2, POFF2, bb2)
    for r in results:
        tot += np.sqrt(np.maximum(
            r['o_cfx'].astype(np.float64).reshape(-1), 0.0)).sum()
        tot += np.sqrt(np.maximum(
            r['o_cgy'].astype(np.float64).reshape(-1), 0.0)).sum()
    return np.array(tot / float(B), dtype=np.float32)


def kernel(**inputs):
    from concourse.bass_utils import run_bass_kernel_spmd

    if 'nc' not in _CACHE:
        _CACHE['nc'] = _build_nc()
    nc = _CACHE['nc']
    in_maps, bb1, bb2 = _host_prep(inputs)
    res = run_bass_kernel_spmd(nc, in_maps, core_ids=list(range(8)),
                               trace=TRACE)
    if TRACE and res.exec_time_ns is not None:
        print(f"HW exec time: {res.exec_time_ns} ns")
        _CACHE['last_exec_ns'] = res.exec_time_ns
        _CACHE['last_trace'] = res.instructions_and_trace
    return _host_combine(res.results, bb1, bb2)


# revision 11
# speedup vs baseline: 1.0806x; 1.0806x over previous
"""Trainium2 Bass kernel for nn_AlignedGloveLayer (retrieval_knn).

TimelineSim per-core estimate 128.6us (prior baseline 207.5us); hardware
rel err vs the fp32 jax reference ~1.3e-4.

Sharding (8 NeuronCores, SPMD): each core runs the small MLPs for its own
1024 queries, holds all 8192 check rows as fp8 stationaries, and emits
per-check-row statistics over its query range; the host min/softmin-combines
the 8 shards.

Per-core engine plan:
  - check-row norms (bb) are host-side input preprocessing.
  - The 128 cdist tiles' [128, 1024] psum reductions split across two lanes:
    DVE tensor_reduce(min) for NV1+NV2 tiles, ACT Exp softmin (in-place on
    psum, accum_out) for the rest; device pivots for both cdists.
  - +aa[i]/+gg[i] via one fp8 DoubleRow K=2 matmul per psum half (aa split
    hi+lo fp8 rows for precision).
  - Cycle-consistency reuses bf16 copies A_bf/G_bf of the MLP outputs; the
    (W2 h + b - x) difference is accumulated in psum via -identity and
    bias-outer-product matmuls, squared on ACT.
  - All small constants ride in two packed blob DMAs (HWDGE is serial at
    ~625ns/DMA); yc/xc stationaries stream in 4 chunks each.
"""

import numpy as np
import ml_dtypes

BF = ml_dtypes.bfloat16
F32 = np.float32
F8 = ml_dtypes.float8_e4m3

B = 8192          # query batch
S = B // 8        # per-core query shard
DX, DY, H = 512, 256, 100
P = 128
GX, GY = DX // P, DY // P   # 4, 2 contraction groups
MX, MY = DX // P, DY // P

BETA1, POFF1 = 25.0, 2.5
BETA2, POFF2 = 20.0, 3.0

# lane maps: True -> DVE tensor_reduce (exact min), False -> ACT softmin.
# cdist1: evens + the early odd tiles (ACT is busy with the MLP prologue
# when jt 1..7 drain); cdist2: odds. Strict v,a alternation mid-kernel.
XTRA1 = (1, 3, 5, 7)     # early odds -> DVE (ACT busy with prologue)
FLIP1 = (2, 6)                # evens -> ACT (balance knob)

# bf16 const blob column offsets
CW_FX1 = 0            # [128, 4*100]
CW_GY1 = 400          # [128, 2*100]
CW_FX2 = 600          # [100, 256]
CW_GY2 = 856          # [100, 512]
C_ONES = 1368         # [128, 128]
C_NEGI = 1496         # [128, 128]
C_ONESR = 1624        # [1, 512]
C_FXB2T = 2136        # [1, 256]
C_GYB2T = 2392        # [1, 512]
NBLOB = 2904


LANE1 = [((jt % 2 == 0) or (jt in XTRA1)) and (jt not in FLIP1)
         for jt in range(64)]
LANE2 = [(jt % 2 == 1) for jt in range(64)]

TRACE = False
_CACHE = {}


def _legalize_sync(nc, max_total=2, max_ev_waits=2):
    """This container's walrus build rejects instructions carrying more than
    one sync wait (and ~2 sync commands total). Tile attaches full
    vector-clock waits to instructions, so split excess waits onto preceding
    same-engine InstEventSemaphore instructions — engine streams execute in
    order, so a wait executed earlier on the same engine preserves every
    happens-before edge."""
    import concourse.mybir as mybir

    n_new = 0
    for f in nc.m.functions:
        for blk in f.blocks:
            insts = blk.instructions
            need = False
            for inst in insts:
                si = inst.sync_info
                if si is not None and len(si.on_wait) > max(
                        0, min(1, max_total - len(si.on_update))):
                    need = True
                    break
            if not need:
                continue
            out = []
            for inst in insts:
                si = inst.sync_info
                if si is not None:
                    waits = list(si.on_wait)
                    ups = list(si.on_update)
                    assert len(ups) <= max_total, (
                        f"{inst.name}: {len(ups)} sync updates, cannot legalize")
                    keep_w = max(0, min(1, max_total - len(ups)))
                    if len(waits) > keep_w:
                        spill = waits[:len(waits) - keep_w]
                        kept = waits[len(waits) - keep_w:]
                        for k in range(0, len(spill), max_ev_waits):
                            ev = mybir.InstEventSemaphore(
                                name=f"legalw-{nc.next_id()}",
                                engine=inst.engine,
                                ins=[], outs=[],
                                sync_info=mybir.SyncInfo(
                                    on_wait=spill[k:k + max_ev_waits],
                                    on_update=[]),
                            )
                            nc.register_instruction(ev)
                            out.append(ev)
                            n_new += 1
                        inst.sync_info = mybir.SyncInfo(
                            on_wait=kept, on_update=ups)
                out.append(inst)
            blk.instructions = out
    return n_new


def _build_nc():
    import concourse.bass as bass
    import concourse.mybir as mybir
    from concourse.tile import TileContext

    f32 = mybir.dt.float32
    bf16 = mybir.dt.bfloat16
    fp8 = mybir.dt.float8e4
    AF = mybir.ActivationFunctionType
    OP = mybir.AluOpType
    AX = mybir.AxisListType
    DR = mybir.MatmulPerfMode.DoubleRow

    nc = bass.Bass()
    ts = bass.ts

    # ---- DRAM I/O ----
    blobB = nc.dram_tensor("blobB", [P, NBLOB], bf16, kind="ExternalInput")
    blobF = nc.dram_tensor("blobF", [P, 8], f32, kind="ExternalInput")
    ones2in = nc.dram_tensor("ones2in", [1, 2, P], fp8, kind="ExternalInput")
    xpTsb = nc.dram_tensor("xpTsb", [DX, S], bf16, kind="ExternalInput")
    ypTsb = nc.dram_tensor("ypTsb", [DY, S], bf16, kind="ExternalInput")
    ycT2 = nc.dram_tensor("ycT2", [DY, B], fp8, kind="ExternalInput")  # -2*Yc^T
    xcT2 = nc.dram_tensor("xcT2", [DX, B], fp8, kind="ExternalInput")  # -2*Xc^T

    o_min1 = nc.dram_tensor("o_min1", [P, 64], f32, kind="ExternalOutput")
    o_min2 = nc.dram_tensor("o_min2", [P, 64], f32, kind="ExternalOutput")
    o_cfx = nc.dram_tensor("o_cfx", [1, S], f32, kind="ExternalOutput")
    o_cgy = nc.dram_tensor("o_cgy", [1, S], f32, kind="ExternalOutput")
    o_c1 = nc.dram_tensor("o_c1", [1, 1], f32, kind="ExternalOutput")
    o_c2 = nc.dram_tensor("o_c2", [1, 1], f32, kind="ExternalOutput")
    c1_dram = nc.dram_tensor("c1_dram", [1, 1], f32)
    c2_dram = nc.dram_tensor("c2_dram", [1, 1], f32)

    xpTsb_v = xpTsb[:].rearrange("(g p) n -> p g n", p=P)
    ypTsb_v = ypTsb[:].rearrange("(g p) n -> p g n", p=P)
    ycT2_v = ycT2[:].rearrange("(g p) n -> p g n", p=P)
    xcT2_v = xcT2[:].rearrange("(g p) n -> p g n", p=P)

    with TileContext(nc) as tc:
        with (
            tc.tile_pool(name="cpool", bufs=1) as cpool,
        ):
            # ---- ACT warmup: wait-free instructions for table loads ----
            warm = cpool.tile([1, 2], bf16, name="warm")
            nc.vector.memset(warm, 0.0)
            nc.scalar.activation(warm, warm, AF.Exp)
            nc.scalar.copy(warm, warm)
            nc.scalar.activation(warm, warm, AF.Relu)
            nc.scalar.activation(warm, warm, AF.Identity)
            nc.scalar.activation(warm, warm, AF.Square)

            # ---- inputs (HWDGE is serial: order = priority) ----
            blb = cpool.tile([P, NBLOB], bf16, name="blb")
            t_xpsb = cpool.tile([P, GX, S], bf16, name="t_xpsb")
            nc.sync.dma_start(out=t_xpsb[:, :, 0:512],
                              in_=xpTsb_v[:, :, 0:512])
            nc.sync.dma_start(out=blb[:, 0:CW_FX2],
                              in_=blobB[:, 0:CW_FX2])
            blf = cpool.tile([P, 8], f32, name="blf")
            nc.sync.dma_start(out=blf, in_=blobF[:])
            nc.sync.dma_start(out=blb[:, CW_FX2:C_ONES + P],
                              in_=blobB[:, CW_FX2:C_ONES + P])
            nc.sync.dma_start(out=t_xpsb[:, :, 512:1024],
                              in_=xpTsb_v[:, :, 512:1024])
            t_ypsb = cpool.tile([P, GY, S], bf16, name="t_ypsb")
            nc.sync.dma_start(out=t_ypsb, in_=ypTsb_v)
            nc.sync.dma_start(out=blb[:, C_ONES + P:NBLOB],
                              in_=blobB[:, C_ONES + P:NBLOB])
            ones2 = cpool.tile([1, 2, P], fp8, name="ones2")
            nc.sync.dma_start(out=ones2, in_=ones2in[:])
            t_yc = cpool.tile([P, GY, B], fp8, name="t_yc")
            t_xc = cpool.tile([P, GX, B], fp8, name="t_xc")
            for ch in range(4):
                chs = ts(ch, B // 4)
                nc.sync.dma_start(out=t_yc[:, :, chs], in_=ycT2_v[:, :, chs])
            for ch in range(4):
                chs = ts(ch, B // 4)
                nc.sync.dma_start(out=t_xc[:, :, chs], in_=xcT2_v[:, :, chs])

            # blob views
            def w_fx1(g):
                return blb[:, CW_FX1 + g * H:CW_FX1 + (g + 1) * H]

            def w_gy1(g):
                return blb[:, CW_GY1 + g * H:CW_GY1 + (g + 1) * H]

            w_fx2 = blb[0:H, CW_FX2:CW_FX2 + DY]
            w_gy2 = blb[0:H, CW_GY2:CW_GY2 + DX]
            onescol = blb[:, C_ONES:C_ONES + 1]
            negI = blb[:, C_NEGI:C_NEGI + P]
            onesr = blb[0:1, C_ONESR:C_ONESR + 512]
            fxb2T = blb[0:1, C_FXB2T:C_FXB2T + DY]
            gyb2T = blb[0:1, C_GYB2T:C_GYB2T + DX]
            b_fx1 = blf[0:H, 0:1]
            b_gy1 = blf[0:H, 1:2]

            def b_fx2(mg):
                return blf[:, 2 + mg:3 + mg]

            def b_gy2(mg):
                return blf[:, 4 + mg:5 + mg]

            aarow = cpool.tile([1, S], f32, name="aarow")
            ggrow = cpool.tile([1, S], f32, name="ggrow")
            aahl = cpool.tile([1, 2, S], fp8, name="aahl")
            gghl = cpool.tile([1, 2, S], fp8, name="gghl")
            A_loc = cpool.tile([P, MY, S], fp8, name="A_loc")
            A_bf = cpool.tile([P, MY, S], bf16, name="A_bf")
            G_loc = cpool.tile([P, MX, S], fp8, name="G_loc")
            G_bf = cpool.tile([P, MX, S], bf16, name="G_bf")
            o1_sb = cpool.tile([P, 64], f32, name="o1_sb")
            o2_sb = cpool.tile([P, 64], f32, name="o2_sb")
            bias1 = cpool.tile([P, 1], f32, name="bias1")
            bias2 = cpool.tile([P, 1], f32, name="bias2")
            sqA = cpool.tile([P, MY, S], bf16, name="sqA")
            sqG = cpool.tile([P, MX, S], bf16, name="sqG")
            dsqX = cpool.tile([P, MX, S], bf16, name="dsqX")
            dsqY = cpool.tile([P, MY, S], bf16, name="dsqY")
            h4P = cpool.tile([H, 4, 512], bf16, name="h4P")

            with (
                tc.tile_pool(name="spool", bufs=2) as spool,
            ):
                psp = tc.alloc_tile_pool(name="psp", bufs=4, space="PSUM")

                def emit_fx_mlp(nst, nsub=1):
                    # nsub=2 splits into 256-col substeps: shorter first-A
                    # latency for the aa critical path in the prologue
                    w = 512 // nsub
                    for hh in range(nsub):
                        sl = bass.ds(nst * 512 + hh * w, w)
                        ps_h = psp.tile([H, w], f32, name="ps_h", tag="mix",
                                        bufs=4)
                        for g in range(GX):
                            nc.tensor.matmul(ps_h, w_fx1(g),
                                             t_xpsb[:, g, sl],
                                             start=(g == 0),
                                             stop=(g == GX - 1))
                        h_sb = spool.tile([H, w], bf16, name="h_sb",
                                          tag="h_sb")
                        nc.scalar.activation(h_sb, ps_h, AF.Relu, bias=b_fx1)
                        for mg in range(MY):
                            ps_a = psp.tile([P, w], f32, name="ps_a",
                                            tag="mix", bufs=4)
                            nc.tensor.matmul(ps_a, w_fx2[:, ts(mg, P)],
                                             h_sb, start=True, stop=True)
                            nc.scalar.activation(A_loc[:, mg, sl], ps_a,
                                                 AF.Identity, bias=b_fx2(mg))
                            nc.vector.tensor_scalar(A_bf[:, mg, sl], ps_a,
                                                    b_fx2(mg), None, OP.add)
                            nc.vector.tensor_tensor(sqA[:, mg, sl],
                                                    A_bf[:, mg, sl],
                                                    A_bf[:, mg, sl],
                                                    OP.mult)

                def emit_fx_aa(nst):
                    sl = ts(nst, 512)
                    ps_aa = psp.tile([1, 512], f32, name="ps_aa", tag="mix",
                                     bufs=4)
                    for mg in range(MY):
                        nc.tensor.matmul(ps_aa, onescol, sqA[:, mg, sl],
                                         start=(mg == 0), stop=(mg == MY - 1))
                    nc.scalar.copy(aarow[0:1, sl], ps_aa)
                    nc.gpsimd.tensor_copy(aahl[0:1, 0, sl], aarow[0:1, sl])
                    nc.gpsimd.tensor_tensor(aahl[0:1, 1, sl], aarow[0:1, sl],
                                            aahl[0:1, 0, sl], OP.subtract)

                def emit_gy_mlp(nst):
                    sl = ts(nst, 512)
                    ps_h2 = psp.tile([H, 512], f32, name="ps_h2", tag="mix",
                                     bufs=4)
                    for g in range(GY):
                        nc.tensor.matmul(ps_h2, w_gy1(g), t_ypsb[:, g, sl],
                                         start=(g == 0), stop=(g == GY - 1))
                    h2_sb = spool.tile([H, 512], bf16, name="h2_sb",
                                       tag="h_sb")
                    nc.scalar.activation(h2_sb, ps_h2, AF.Relu, bias=b_gy1)
                    for mg in range(MX):
                        ps_g = psp.tile([P, 512], f32, name="ps_g",
                                        tag="mix", bufs=4)
                        nc.tensor.matmul(ps_g, w_gy2[:, ts(mg, P)],
                                         h2_sb, start=True, stop=True)
                        nc.scalar.activation(G_loc[:, mg, sl], ps_g,
                                             AF.Identity, bias=b_gy2(mg))
                        nc.vector.tensor_scalar(G_bf[:, mg, sl], ps_g,
                                                b_gy2(mg), None, OP.add)
                        nc.vector.tensor_tensor(sqG[:, mg, sl],
                                                G_bf[:, mg, sl],
                                                G_bf[:, mg, sl], OP.mult)

                def emit_gy_gg(nst):
                    sl = ts(nst, 512)
                    ps_gg = psp.tile([1, 512], f32, name="ps_gg", tag="mix",
                                     bufs=4)
                    for mg in range(MX):
                        nc.tensor.matmul(ps_gg, onescol, sqG[:, mg, sl],
                                         start=(mg == 0), stop=(mg == MX - 1))
                    nc.scalar.copy(ggrow[0:1, sl], ps_gg)
                    nc.gpsimd.tensor_copy(gghl[0:1, 0, sl], ggrow[0:1, sl])
                    nc.gpsimd.tensor_tensor(gghl[0:1, 1, sl], ggrow[0:1, sl],
                                            gghl[0:1, 0, sl], OP.subtract)

                cbf_t = {}

                def post_min(row, o_c, key):
                    # pivot = min(row); broadcast deferred (see post_bias)
                    c_sb = spool.tile([1, 1], f32, name="c_sb", tag="c_sb")
                    nc.vector.tensor_reduce(c_sb, row, axis=AX.X, op=OP.min)
                    nc.sync.dma_start(out=o_c[:], in_=c_sb)
                    c_bf = spool.tile([1, 1], bf16, name="c_bf", tag="cbf",
                                      bufs=2)
                    nc.vector.tensor_copy(c_bf, c_sb)
                    cbf_t[key] = c_bf

                def post_bias(key, biasT, beta, poff):
                    # [P,1] pivot broadcast via 1-element matmul; emitted
                    # late enough that PE reaches it after the pivot chain
                    ps_b = psp.tile([P, 1], f32, name="ps_b", tag="mix",
                                    bufs=4)
                    nc.tensor.matmul(ps_b, blb[0:1, C_ONES:C_ONES + P],
                                     cbf_t[key], start=True, stop=True)
                    nc.vector.tensor_scalar(biasT, ps_b, beta, -beta * poff,
                                            OP.mult, OP.add)

                def emit_cd_tile(which, jt):
                    t_st, m_sb, hl, npair, o_sb, biasT, beta, lane = (
                        (t_yc, A_loc, aahl, 1, o1_sb, bias1, BETA1,
                         LANE1[jt]) if which == 0 else
                        (t_xc, G_loc, gghl, 2, o2_sb, bias2, BETA2,
                         LANE2[jt]))
                    jsl = ts(jt, P)
                    ps = psp.tile([P, 1024], f32, name="ps_cd", tag="mix",
                                  bufs=4)
                    for h in range(2):
                        isl = ts(h, 512)
                        ph = ps[:, ts(h, 512)]
                        for pr in range(npair):
                            nc.tensor.matmul(
                                ph, t_st[:, 2 * pr:2 * pr + 2, jsl],
                                m_sb[:, 2 * pr:2 * pr + 2, isl],
                                start=(pr == 0), stop=False, perf_mode=DR)
                        nc.tensor.matmul(ph, ones2, hl[:, :, isl],
                                         start=False, stop=True, perf_mode=DR)
                    if lane:
                        nc.vector.tensor_reduce(o_sb[:, jt:jt + 1], ps,
                                                axis=AX.X, op=OP.min)
                    else:
                        nc.scalar.activation(ps, ps, AF.Exp, bias=biasT,
                                             scale=-beta,
                                             accum_out=o_sb[:, jt:jt + 1])

                def emit_cycle_head(side, nst):
                    # side 0: x->y->x second stage gy(A_bf); side 1: fx(G_bf)
                    csl = ts(nst, 512)
                    w1, src_bf, b1h, ng = (
                        (w_gy1, A_bf, b_gy1, GY) if side == 0 else
                        (w_fx1, G_bf, b_fx1, GX))
                    ps_h4 = psp.tile([H, 512], f32, name="ps_h4",
                                     tag="mix", bufs=4)
                    for g in range(ng):
                        nc.tensor.matmul(ps_h4, w1(g), src_bf[:, g, csl],
                                         start=(g == 0), stop=(g == ng - 1))
                    nc.scalar.activation(h4P[:, 2 * side + nst, :], ps_h4,
                                         AF.Relu, bias=b1h)

                def emit_cycle_mgs(side, nst, mgl):
                    csl = ts(nst, 512)
                    w2, bT, xin, dsq_t = (
                        (w_gy2, gyb2T, t_xpsb, dsqX) if side == 0 else
                        (w_fx2, fxb2T, t_ypsb, dsqY))
                    h4_sb = h4P[:, 2 * side + nst, :]
                    for mg in mgl:
                        # psum accumulates W2 h + b - x; ACT squares it
                        ps_xr = psp.tile([P, 512], f32, name="ps_xr",
                                         tag="mix", bufs=4)
                        nc.tensor.matmul(ps_xr, w2[:, ts(mg, P)], h4_sb,
                                         start=True, stop=False)
                        nc.tensor.matmul(ps_xr, negI, xin[:, mg, csl],
                                         start=False, stop=False)
                        nc.tensor.matmul(ps_xr, bT[0:1, ts(mg, P)], onesr,
                                         start=False, stop=True)
                        nc.scalar.activation(dsq_t[:, mg, csl], ps_xr,
                                             AF.Square)

                def emit_cycle_sum(side, nst):
                    csl = ts(nst, 512)
                    nmg, dsq_t, o_c = ((MX, dsqX, o_cfx) if side == 0
                                       else (MY, dsqY, o_cgy))
                    ps_nfx = psp.tile([1, 512], f32, name="ps_nfx",
                                      tag="mix", bufs=4)
                    for mg in range(nmg):
                        nc.tensor.matmul(ps_nfx, onescol, dsq_t[:, mg, csl],
                                         start=(mg == 0), stop=(mg == nmg - 1))
                    st_fx = spool.tile([1, 512], f32, name="st_fx",
                                       tag="stage")
                    nc.scalar.copy(st_fx, ps_nfx)
                    nc.sync.dma_start(out=o_c[0:1, csl], in_=st_fx)

                # ---- schedule ----
                wmm = spool.tile([P, 512], bf16, name="wmm", bufs=1)
                nc.vector.memset(wmm, 0.0)
                for _ in range(2):
                    wps = psp.tile([P, 512], f32, name="wps", tag="mix",
                                   bufs=4)
                    nc.tensor.matmul(wps, wmm[:, 0:P], wmm,
                                     start=True, stop=True)
                emit_fx_mlp(0)
                emit_fx_mlp(1)
                emit_gy_mlp(0)
                emit_fx_aa(0)
                emit_fx_aa(1)
                post_min(aarow, o_c1, 1)
                emit_gy_mlp(1)
                emit_gy_gg(0)
                emit_gy_gg(1)
                post_min(ggrow, o_c2, 2)
                emit_cd_tile(0, 0)
                post_bias(1, bias1, BETA1, POFF1)
                emit_cd_tile(0, 1)
                emit_cd_tile(0, 2)
                post_bias(2, bias2, BETA2, POFF2)
                emit_cd_tile(0, 3)
                emit_cycle_head(0, 0)
                emit_cd_tile(0, 4)
                emit_cycle_head(0, 1)
                emit_cd_tile(0, 5)
                emit_cycle_head(1, 0)
                emit_cd_tile(0, 6)
                emit_cycle_head(1, 1)
                emit_cd_tile(0, 7)
                # (side, nst, mg chunk) trickled through the main loop
                cjobs = {9: (0, 0, (0, 1)), 13: (0, 0, (2, 3)),
                         17: (0, 1, (0, 1)), 21: (0, 1, (2, 3)),
                         25: (1, 0, (0, 1)), 29: (1, 1, (0, 1))}
                csums = {15: (0, 0), 23: (0, 1), 31: (1, 0), 35: (1, 1)}
                for jt in range(8, 64):
                    # DVE-lane member of the pair first
                    if LANE1[jt] or not LANE2[jt - 8]:
                        emit_cd_tile(0, jt)
                        emit_cd_tile(1, jt - 8)
                    else:
                        emit_cd_tile(1, jt - 8)
                        emit_cd_tile(0, jt)
                    if jt in cjobs:
                        emit_cycle_mgs(*cjobs[jt])
                    if jt in csums:
                        emit_cycle_sum(*csums[jt])
                    if jt == 55:
                        nc.sync.dma_start(out=o_min2[:, 0:48],
                                          in_=o2_sb[:, 0:48])
                nc.sync.dma_start(out=o_min1[:], in_=o1_sb)
                for jt in range(56, 64):
                    emit_cd_tile(1, jt)
                psp.release()
                nc.sync.dma_start(out=o_min2[:, 48:64], in_=o2_sb[:, 48:64])

    _legalize_sync(nc)
    nc.finalize()
    return nc


def _host_prep(inputs):
    """Gather/transpose/cast on host -> per-core input maps + bb norms."""
    xw = np.asarray(inputs['x_weight'], dtype=np.float32)
    yw = np.asarray(inputs['y_weight'], dtype=np.float32)
    xp = np.asarray(inputs['x_present']).astype(np.int64)
    yc = np.asarray(inputs['y_check']).astype(np.int64)
    yp = np.asarray(inputs['y_present']).astype(np.int64)
    xc = np.asarray(inputs['x_check']).astype(np.int64)

    def c(a, dt):
        return np.ascontiguousarray(a, dtype=dt)

    # packed bf16 const blob
    blobB = np.zeros((P, NBLOB), dtype=BF)
    fxW1 = np.asarray(inputs['fx_W1'], dtype=np.float32)  # [DX, H]
    gyW1 = np.asarray(inputs['gy_W1'], dtype=np.float32)  # [DY, H]
    for g in range(GX):
        blobB[:, CW_FX1 + g * H:CW_FX1 + (g + 1) * H] = \
            fxW1[g * P:(g + 1) * P, :].astype(BF)
    for g in range(GY):
        blobB[:, CW_GY1 + g * H:CW_GY1 + (g + 1) * H] = \
            gyW1[g * P:(g + 1) * P, :].astype(BF)
    blobB[0:H, CW_FX2:CW_FX2 + DY] = np.asarray(inputs['fx_W2']).astype(BF)
    blobB[0:H, CW_GY2:CW_GY2 + DX] = np.asarray(inputs['gy_W2']).astype(BF)
    blobB[:, C_ONES:C_ONES + P] = np.ones((P, P), dtype=BF)
    blobB[:, C_NEGI:C_NEGI + P] = (-np.eye(P)).astype(BF)
    blobB[0:1, C_ONESR:C_ONESR + 512] = np.ones((1, 512), dtype=BF)
    blobB[0:1, C_FXB2T:C_FXB2T + DY] = \
        np.asarray(inputs['fx_b2']).reshape(1, -1).astype(BF)
    blobB[0:1, C_GYB2T:C_GYB2T + DX] = \
        np.asarray(inputs['gy_b2']).reshape(1, -1).astype(BF)

    blobF = np.zeros((P, 8), dtype=F32)
    blobF[0:H, 0] = np.asarray(inputs['fx_b1']).reshape(-1)
    blobF[0:H, 1] = np.asarray(inputs['gy_b1']).reshape(-1)
    blobF[:, 2:4] = np.asarray(inputs['fx_b2']).reshape(MY, P).T
    blobF[:, 4:8] = np.asarray(inputs['gy_b2']).reshape(MX, P).T

    ycr = yw[yc]
    xcr = xw[xc]
    shared = {
        'blobB': blobB,
        'blobF': blobF,
        'ones2in': np.ones((1, 2, P), dtype=F8),
        'ycT2': c(-2.0 * ycr.T, F8),
        'xcT2': c(-2.0 * xcr.T, F8),
    }
    in_maps = []
    for cix in range(8):
        sl = slice(cix * S, (cix + 1) * S)
        m = dict(shared)
        m['xpTsb'] = c(xw[xp[sl]].T, BF)
        m['ypTsb'] = c(yw[yp[sl]].T, BF)
        in_maps.append(m)
    bb1 = np.sum(ycr.astype(np.float64) ** 2, axis=1)
    bb2 = np.sum(xcr.astype(np.float64) ** 2, axis=1)
    return in_maps, bb1, bb2


def _combine_side(results, key, ckey, lanes, beta, poff, bb):
    """Column-combine one cdist: exact-min cols by min over shards, softmin
    cols by pivot-rescaled sumexp; then +bb, clamp, sqrt, sum."""
    # device broadcasts the pivot through a bf16 1-element matmul;
    # replicate that rounding exactly
    pivots = [float(np.float32(r[ckey][0, 0]).astype(BF)) - poff
              for r in results]
    cstar = min(pivots)
    stot = np.zeros((P, 64), np.float64)
    for r, pv in zip(results, pivots):
        stot += r[key].astype(np.float64) * np.exp(beta * (cstar - pv))
    soft = cstar - np.log(np.maximum(stot, 1e-300)) / beta
    hard = np.min(np.stack([r[key] for r in results]),
                  axis=0).astype(np.float64)
    comb = np.where(np.asarray(lanes)[None, :], hard, soft)
    d = comb.T.reshape(-1) + bb
    return np.sqrt(np.maximum(d, 0.0)).sum()


def _host_combine(results, bb1, bb2):
    tot = _combine_side(results, 'o_min1', 'o_c1', LANE1, BETA1, POFF1, bb1)
    tot += _combine_side(results, 'o_min2', 'o_c2', LANE2, BETA2, POFF2, bb2)
    for r in results:
        tot += np.sqrt(np.maximum(
            r['o_cfx'].astype(np.float64).reshape(-1), 0.0)).sum()
        tot += np.sqrt(np.maximum(
            r['o_cgy'].astype(np.float64).reshape(-1), 0.0)).sum()
    return np.array(tot / float(B), dtype=np.float32)


def kernel(**inputs):
    from concourse.bass_utils import run_bass_kernel_spmd

    if 'nc' not in _CACHE:
        _CACHE['nc'] = _build_nc()
    nc = _CACHE['nc']
    in_maps, bb1, bb2 = _host_prep(inputs)
    res = run_bass_kernel_spmd(nc, in_maps, core_ids=list(range(8)),
                               trace=TRACE)
    if TRACE and res.exec_time_ns is not None:
        print(f"HW exec time: {res.exec_time_ns} ns")
        _CACHE['last_exec_ns'] = res.exec_time_ns
        _CACHE['last_trace'] = res.instructions_and_trace
    return _host_combine(res.results, bb1, bb2)


# revision 12
# speedup vs baseline: 1.0833x; 1.0025x over previous
"""Trainium2 Bass kernel for nn_AlignedGloveLayer (retrieval_knn).

TimelineSim per-core estimate 128.6us (prior baseline 207.5us); hardware
rel err vs the fp32 jax reference ~1.3e-4.

Sharding (8 NeuronCores, SPMD): each core runs the small MLPs for its own
1024 queries, holds all 8192 check rows as fp8 stationaries, and emits
per-check-row statistics over its query range; the host min/softmin-combines
the 8 shards.

Per-core engine plan:
  - check-row norms (bb) are host-side input preprocessing.
  - The 128 cdist tiles' [128, 1024] psum reductions split across two lanes:
    DVE tensor_reduce(min) for NV1+NV2 tiles, ACT Exp softmin (in-place on
    psum, accum_out) for the rest; device pivots for both cdists.
  - +aa[i]/+gg[i] via one fp8 DoubleRow K=2 matmul per psum half (aa split
    hi+lo fp8 rows for precision).
  - Cycle-consistency reuses bf16 copies A_bf/G_bf of the MLP outputs; the
    (W2 h + b - x) difference is accumulated in psum via -identity and
    bias-outer-product matmuls, squared on ACT.
  - All small constants ride in two packed blob DMAs (HWDGE is serial at
    ~625ns/DMA); yc/xc stationaries stream in 4 chunks each.
"""

import numpy as np
import ml_dtypes

BF = ml_dtypes.bfloat16
F32 = np.float32
F8 = ml_dtypes.float8_e4m3

B = 8192          # query batch
S = B // 8        # per-core query shard
DX, DY, H = 512, 256, 100
P = 128
GX, GY = DX // P, DY // P   # 4, 2 contraction groups
MX, MY = DX // P, DY // P

BETA1, POFF1 = 25.0, 2.5
BETA2, POFF2 = 20.0, 3.0

# lane maps: True -> DVE tensor_reduce (exact min), False -> ACT softmin.
# cdist1: evens + the early odd tiles (ACT is busy with the MLP prologue
# when jt 1..7 drain); cdist2: odds. Strict v,a alternation mid-kernel.
XTRA1 = (1, 3, 5, 7)     # early odds -> DVE (ACT busy with prologue)
FLIP1 = (2, 6)                # evens -> ACT (balance knob)

# bf16 const blob column offsets
CW_FX1 = 0            # [128, 4*100]
CW_GY1 = 400          # [128, 2*100]
CW_FX2 = 600          # [100, 256]
CW_GY2 = 856          # [100, 512]
C_ONES = 1368         # [128, 128]
C_NEGI = 1496         # [128, 128]
C_ONESR = 1624        # [1, 512]
C_FXB2T = 2136        # [1, 256]
C_GYB2T = 2392        # [1, 512]
NBLOB = 2904


LANE1 = [((jt % 2 == 0) or (jt in XTRA1)) and (jt not in FLIP1)
         for jt in range(64)]
LANE2 = [(jt % 2 == 1) for jt in range(64)]

TRACE = False
_CACHE = {}


def _legalize_sync(nc, max_total=2, max_ev_waits=2):
    """This container's walrus build rejects instructions carrying more than
    one sync wait (and ~2 sync commands total). Tile attaches full
    vector-clock waits to instructions, so split excess waits onto preceding
    same-engine InstEventSemaphore instructions — engine streams execute in
    order, so a wait executed earlier on the same engine preserves every
    happens-before edge."""
    import concourse.mybir as mybir

    n_new = 0
    for f in nc.m.functions:
        for blk in f.blocks:
            insts = blk.instructions
            need = False
            for inst in insts:
                si = inst.sync_info
                if si is not None and len(si.on_wait) > max(
                        0, min(1, max_total - len(si.on_update))):
                    need = True
                    break
            if not need:
                continue
            out = []
            for inst in insts:
                si = inst.sync_info
                if si is not None:
                    waits = list(si.on_wait)
                    ups = list(si.on_update)
                    assert len(ups) <= max_total, (
                        f"{inst.name}: {len(ups)} sync updates, cannot legalize")
                    keep_w = max(0, min(1, max_total - len(ups)))
                    if len(waits) > keep_w:
                        spill = waits[:len(waits) - keep_w]
                        kept = waits[len(waits) - keep_w:]
                        for k in range(0, len(spill), max_ev_waits):
                            ev = mybir.InstEventSemaphore(
                                name=f"legalw-{nc.next_id()}",
                                engine=inst.engine,
                                ins=[], outs=[],
                                sync_info=mybir.SyncInfo(
                                    on_wait=spill[k:k + max_ev_waits],
                                    on_update=[]),
                            )
                            nc.register_instruction(ev)
                            out.append(ev)
                            n_new += 1
                        inst.sync_info = mybir.SyncInfo(
                            on_wait=kept, on_update=ups)
                out.append(inst)
            blk.instructions = out
    return n_new


def _build_nc():
    import concourse.bass as bass
    import concourse.mybir as mybir
    from concourse.tile import TileContext

    f32 = mybir.dt.float32
    bf16 = mybir.dt.bfloat16
    fp8 = mybir.dt.float8e4
    AF = mybir.ActivationFunctionType
    OP = mybir.AluOpType
    AX = mybir.AxisListType
    DR = mybir.MatmulPerfMode.DoubleRow

    nc = bass.Bass()
    ts = bass.ts

    # ---- DRAM I/O ----
    blobB = nc.dram_tensor("blobB", [P, NBLOB], bf16, kind="ExternalInput")
    blobF = nc.dram_tensor("blobF", [P, 8], f32, kind="ExternalInput")
    ones2in = nc.dram_tensor("ones2in", [1, 2, P], fp8, kind="ExternalInput")
    xpTsb = nc.dram_tensor("xpTsb", [DX, S], bf16, kind="ExternalInput")
    ypTsb = nc.dram_tensor("ypTsb", [DY, S], bf16, kind="ExternalInput")
    ycT2 = nc.dram_tensor("ycT2", [DY, B], fp8, kind="ExternalInput")  # -2*Yc^T
    xcT2 = nc.dram_tensor("xcT2", [DX, B], fp8, kind="ExternalInput")  # -2*Xc^T

    o_min1 = nc.dram_tensor("o_min1", [P, 64], f32, kind="ExternalOutput")
    o_min2 = nc.dram_tensor("o_min2", [P, 64], f32, kind="ExternalOutput")
    o_cfx = nc.dram_tensor("o_cfx", [1, S], f32, kind="ExternalOutput")
    o_cgy = nc.dram_tensor("o_cgy", [1, S], f32, kind="ExternalOutput")
    o_c1 = nc.dram_tensor("o_c1", [1, 1], f32, kind="ExternalOutput")
    o_c2 = nc.dram_tensor("o_c2", [1, 1], f32, kind="ExternalOutput")
    c1_dram = nc.dram_tensor("c1_dram", [1, 1], f32)
    c2_dram = nc.dram_tensor("c2_dram", [1, 1], f32)

    xpTsb_v = xpTsb[:].rearrange("(g p) n -> p g n", p=P)
    ypTsb_v = ypTsb[:].rearrange("(g p) n -> p g n", p=P)
    ycT2_v = ycT2[:].rearrange("(g p) n -> p g n", p=P)
    xcT2_v = xcT2[:].rearrange("(g p) n -> p g n", p=P)

    with TileContext(nc) as tc:
        with (
            tc.tile_pool(name="cpool", bufs=1) as cpool,
        ):
            # ---- ACT warmup: wait-free instructions for table loads ----
            warm = cpool.tile([1, 2], bf16, name="warm")
            nc.vector.memset(warm, 0.0)
            nc.scalar.activation(warm, warm, AF.Exp)
            nc.scalar.copy(warm, warm)
            nc.scalar.activation(warm, warm, AF.Relu)
            nc.scalar.activation(warm, warm, AF.Identity)
            nc.scalar.activation(warm, warm, AF.Square)

            # ---- inputs (HWDGE is serial: order = priority) ----
            blb = cpool.tile([P, NBLOB], bf16, name="blb")
            t_xpsb = cpool.tile([P, GX, S], bf16, name="t_xpsb")
            nc.sync.dma_start(out=t_xpsb[:, :, 0:512],
                              in_=xpTsb_v[:, :, 0:512])
            nc.sync.dma_start(out=blb[:, 0:CW_FX2],
                              in_=blobB[:, 0:CW_FX2])
            blf = cpool.tile([P, 8], f32, name="blf")
            nc.sync.dma_start(out=blf, in_=blobF[:])
            nc.sync.dma_start(out=blb[:, CW_FX2:C_ONES + P],
                              in_=blobB[:, CW_FX2:C_ONES + P])
            nc.sync.dma_start(out=t_xpsb[:, :, 512:1024],
                              in_=xpTsb_v[:, :, 512:1024])
            t_ypsb = cpool.tile([P, GY, S], bf16, name="t_ypsb")
            nc.sync.dma_start(out=t_ypsb, in_=ypTsb_v)
            nc.sync.dma_start(out=blb[:, C_ONES + P:NBLOB],
                              in_=blobB[:, C_ONES + P:NBLOB])
            ones2 = cpool.tile([1, 2, P], fp8, name="ones2")
            nc.sync.dma_start(out=ones2, in_=ones2in[:])
            t_yc = cpool.tile([P, GY, B], fp8, name="t_yc")
            t_xc = cpool.tile([P, GX, B], fp8, name="t_xc")
            for ch in range(4):
                chs = ts(ch, B // 4)
                nc.sync.dma_start(out=t_yc[:, :, chs], in_=ycT2_v[:, :, chs])
            for ch in range(4):
                chs = ts(ch, B // 4)
                nc.sync.dma_start(out=t_xc[:, :, chs], in_=xcT2_v[:, :, chs])

            # blob views
            def w_fx1(g):
                return blb[:, CW_FX1 + g * H:CW_FX1 + (g + 1) * H]

            def w_gy1(g):
                return blb[:, CW_GY1 + g * H:CW_GY1 + (g + 1) * H]

            w_fx2 = blb[0:H, CW_FX2:CW_FX2 + DY]
            w_gy2 = blb[0:H, CW_GY2:CW_GY2 + DX]
            onescol = blb[:, C_ONES:C_ONES + 1]
            negI = blb[:, C_NEGI:C_NEGI + P]
            onesr = blb[0:1, C_ONESR:C_ONESR + 512]
            fxb2T = blb[0:1, C_FXB2T:C_FXB2T + DY]
            gyb2T = blb[0:1, C_GYB2T:C_GYB2T + DX]
            b_fx1 = blf[0:H, 0:1]
            b_gy1 = blf[0:H, 1:2]

            def b_fx2(mg):
                return blf[:, 2 + mg:3 + mg]

            def b_gy2(mg):
                return blf[:, 4 + mg:5 + mg]

            aarow = cpool.tile([1, S], f32, name="aarow")
            ggrow = cpool.tile([1, S], f32, name="ggrow")
            aahl = cpool.tile([1, 2, S], fp8, name="aahl")
            gghl = cpool.tile([1, 2, S], fp8, name="gghl")
            A_loc = cpool.tile([P, MY, S], fp8, name="A_loc")
            A_bf = cpool.tile([P, MY, S], bf16, name="A_bf")
            G_loc = cpool.tile([P, MX, S], fp8, name="G_loc")
            G_bf = cpool.tile([P, MX, S], bf16, name="G_bf")
            o1_sb = cpool.tile([P, 64], f32, name="o1_sb")
            o2_sb = cpool.tile([P, 64], f32, name="o2_sb")
            bias1 = cpool.tile([P, 1], f32, name="bias1")
            bias2 = cpool.tile([P, 1], f32, name="bias2")
            sqA = cpool.tile([P, MY, S], bf16, name="sqA")
            sqG = cpool.tile([P, MX, S], bf16, name="sqG")
            dsqX = cpool.tile([P, MX, S], bf16, name="dsqX")
            dsqY = cpool.tile([P, MY, S], bf16, name="dsqY")
            h4P = cpool.tile([H, 4, 512], bf16, name="h4P")

            with (
                tc.tile_pool(name="spool", bufs=2) as spool,
            ):
                psp = tc.alloc_tile_pool(name="psp", bufs=4, space="PSUM")

                def emit_fx_mlp(nst, nsub=1):
                    # nsub=2 splits into 256-col substeps: shorter first-A
                    # latency for the aa critical path in the prologue
                    w = 512 // nsub
                    for hh in range(nsub):
                        sl = bass.ds(nst * 512 + hh * w, w)
                        ps_h = psp.tile([H, w], f32, name="ps_h", tag="mix",
                                        bufs=4)
                        for g in range(GX):
                            nc.tensor.matmul(ps_h, w_fx1(g),
                                             t_xpsb[:, g, sl],
                                             start=(g == 0),
                                             stop=(g == GX - 1))
                        h_sb = spool.tile([H, w], bf16, name="h_sb",
                                          tag="h_sb")
                        nc.scalar.activation(h_sb, ps_h, AF.Relu, bias=b_fx1)
                        for mg in range(MY):
                            ps_a = psp.tile([P, w], f32, name="ps_a",
                                            tag="mix", bufs=4)
                            nc.tensor.matmul(ps_a, w_fx2[:, ts(mg, P)],
                                             h_sb, start=True, stop=True)
                            nc.scalar.activation(A_loc[:, mg, sl], ps_a,
                                                 AF.Identity, bias=b_fx2(mg))
                            nc.vector.tensor_scalar(A_bf[:, mg, sl], ps_a,
                                                    b_fx2(mg), None, OP.add)
                            nc.vector.tensor_tensor(sqA[:, mg, sl],
                                                    A_bf[:, mg, sl],
                                                    A_bf[:, mg, sl],
                                                    OP.mult)

                def emit_fx_aa(nst):
                    sl = ts(nst, 512)
                    ps_aa = psp.tile([1, 512], f32, name="ps_aa", tag="mix",
                                     bufs=4)
                    for mg in range(MY):
                        nc.tensor.matmul(ps_aa, onescol, sqA[:, mg, sl],
                                         start=(mg == 0), stop=(mg == MY - 1))
                    nc.scalar.copy(aarow[0:1, sl], ps_aa)
                    nc.gpsimd.tensor_copy(aahl[0:1, 0, sl], aarow[0:1, sl])
                    nc.gpsimd.tensor_tensor(aahl[0:1, 1, sl], aarow[0:1, sl],
                                            aahl[0:1, 0, sl], OP.subtract)

                def emit_gy_mlp(nst):
                    sl = ts(nst, 512)
                    ps_h2 = psp.tile([H, 512], f32, name="ps_h2", tag="mix",
                                     bufs=4)
                    for g in range(GY):
                        nc.tensor.matmul(ps_h2, w_gy1(g), t_ypsb[:, g, sl],
                                         start=(g == 0), stop=(g == GY - 1))
                    h2_sb = spool.tile([H, 512], bf16, name="h2_sb",
                                       tag="h_sb")
                    nc.scalar.activation(h2_sb, ps_h2, AF.Relu, bias=b_gy1)
                    for mg in range(MX):
                        ps_g = psp.tile([P, 512], f32, name="ps_g",
                                        tag="mix", bufs=4)
                        nc.tensor.matmul(ps_g, w_gy2[:, ts(mg, P)],
                                         h2_sb, start=True, stop=True)
                        nc.scalar.activation(G_loc[:, mg, sl], ps_g,
                                             AF.Identity, bias=b_gy2(mg))
                        nc.vector.tensor_scalar(G_bf[:, mg, sl], ps_g,
                                                b_gy2(mg), None, OP.add)
                        nc.vector.tensor_tensor(sqG[:, mg, sl],
                                                G_bf[:, mg, sl],
                                                G_bf[:, mg, sl], OP.mult)

                def emit_gy_gg(nst):
                    sl = ts(nst, 512)
                    ps_gg = psp.tile([1, 512], f32, name="ps_gg", tag="mix",
                                     bufs=4)
                    for mg in range(MX):
                        nc.tensor.matmul(ps_gg, onescol, sqG[:, mg, sl],
                                         start=(mg == 0), stop=(mg == MX - 1))
                    nc.scalar.copy(ggrow[0:1, sl], ps_gg)
                    nc.gpsimd.tensor_copy(gghl[0:1, 0, sl], ggrow[0:1, sl])
                    nc.gpsimd.tensor_tensor(gghl[0:1, 1, sl], ggrow[0:1, sl],
                                            gghl[0:1, 0, sl], OP.subtract)

                cbf_t = {}

                def post_min(row, o_c, key):
                    # pivot = min(row); broadcast deferred (see post_bias)
                    c_sb = spool.tile([1, 1], f32, name="c_sb", tag="c_sb")
                    nc.vector.tensor_reduce(c_sb, row, axis=AX.X, op=OP.min)
                    nc.sync.dma_start(out=o_c[:], in_=c_sb)
                    c_bf = spool.tile([1, 1], bf16, name="c_bf", tag="cbf",
                                      bufs=2)
                    nc.vector.tensor_copy(c_bf, c_sb)
                    cbf_t[key] = c_bf

                def post_bias(key, biasT, beta, poff):
                    # [P,1] pivot broadcast via 1-element matmul; emitted
                    # late enough that PE reaches it after the pivot chain
                    ps_b = psp.tile([P, 1], f32, name="ps_b", tag="mix",
                                    bufs=4)
                    nc.tensor.matmul(ps_b, blb[0:1, C_ONES:C_ONES + P],
                                     cbf_t[key], start=True, stop=True)
                    nc.vector.tensor_scalar(biasT, ps_b, beta, -beta * poff,
                                            OP.mult, OP.add)

                def emit_cd_tile(which, jt):
                    t_st, m_sb, hl, npair, o_sb, biasT, beta, lane = (
                        (t_yc, A_loc, aahl, 1, o1_sb, bias1, BETA1,
                         LANE1[jt]) if which == 0 else
                        (t_xc, G_loc, gghl, 2, o2_sb, bias2, BETA2,
                         LANE2[jt]))
                    jsl = ts(jt, P)
                    ps = psp.tile([P, 1024], f32, name="ps_cd", tag="mix",
                                  bufs=4)
                    for h in range(2):
                        isl = ts(h, 512)
                        ph = ps[:, ts(h, 512)]
                        for pr in range(npair):
                            nc.tensor.matmul(
                                ph, t_st[:, 2 * pr:2 * pr + 2, jsl],
                                m_sb[:, 2 * pr:2 * pr + 2, isl],
                                start=(pr == 0), stop=False, perf_mode=DR)
                        nc.tensor.matmul(ph, ones2, hl[:, :, isl],
                                         start=False, stop=True, perf_mode=DR)
                    if lane:
                        nc.vector.tensor_reduce(o_sb[:, jt:jt + 1], ps,
                                                axis=AX.X, op=OP.min)
                    else:
                        nc.scalar.activation(ps, ps, AF.Exp, bias=biasT,
                                             scale=-beta,
                                             accum_out=o_sb[:, jt:jt + 1])

                def emit_cycle_head(side, nst):
                    # side 0: x->y->x second stage gy(A_bf); side 1: fx(G_bf)
                    csl = ts(nst, 512)
                    w1, src_bf, b1h, ng = (
                        (w_gy1, A_bf, b_gy1, GY) if side == 0 else
                        (w_fx1, G_bf, b_fx1, GX))
                    ps_h4 = psp.tile([H, 512], f32, name="ps_h4",
                                     tag="mix", bufs=4)
                    for g in range(ng):
                        nc.tensor.matmul(ps_h4, w1(g), src_bf[:, g, csl],
                                         start=(g == 0), stop=(g == ng - 1))
                    nc.scalar.activation(h4P[:, 2 * side + nst, :], ps_h4,
                                         AF.Relu, bias=b1h)

                def emit_cycle_mgs(side, nst, mgl):
                    csl = ts(nst, 512)
                    w2, bT, xin, dsq_t = (
                        (w_gy2, gyb2T, t_xpsb, dsqX) if side == 0 else
                        (w_fx2, fxb2T, t_ypsb, dsqY))
                    h4_sb = h4P[:, 2 * side + nst, :]
                    for mg in mgl:
                        # psum accumulates W2 h + b - x; ACT squares it
                        ps_xr = psp.tile([P, 512], f32, name="ps_xr",
                                         tag="mix", bufs=4)
                        nc.tensor.matmul(ps_xr, w2[:, ts(mg, P)], h4_sb,
                                         start=True, stop=False)
                        nc.tensor.matmul(ps_xr, negI, xin[:, mg, csl],
                                         start=False, stop=False)
                        nc.tensor.matmul(ps_xr, bT[0:1, ts(mg, P)], onesr,
                                         start=False, stop=True)
                        nc.scalar.activation(dsq_t[:, mg, csl], ps_xr,
                                             AF.Square)

                def emit_cycle_sum(side, nst):
                    csl = ts(nst, 512)
                    nmg, dsq_t, o_c = ((MX, dsqX, o_cfx) if side == 0
                                       else (MY, dsqY, o_cgy))
                    ps_nfx = psp.tile([1, 512], f32, name="ps_nfx",
                                      tag="mix", bufs=4)
                    for mg in range(nmg):
                        nc.tensor.matmul(ps_nfx, onescol, dsq_t[:, mg, csl],
                                         start=(mg == 0), stop=(mg == nmg - 1))
                    st_fx = spool.tile([1, 512], f32, name="st_fx",
                                       tag="stage")
                    nc.vector.tensor_copy(st_fx, ps_nfx)
                    nc.sync.dma_start(out=o_c[0:1, csl], in_=st_fx)

                # ---- schedule ----
                wmm = spool.tile([P, 512], bf16, name="wmm", bufs=1)
                nc.vector.memset(wmm, 0.0)
                for _ in range(2):
                    wps = psp.tile([P, 512], f32, name="wps", tag="mix",
                                   bufs=4)
                    nc.tensor.matmul(wps, wmm[:, 0:P], wmm,
                                     start=True, stop=True)
                emit_fx_mlp(0)
                emit_fx_mlp(1)
                emit_gy_mlp(0)
                emit_fx_aa(0)
                emit_fx_aa(1)
                post_min(aarow, o_c1, 1)
                emit_gy_mlp(1)
                emit_gy_gg(0)
                emit_gy_gg(1)
                post_min(ggrow, o_c2, 2)
                emit_cd_tile(0, 0)
                post_bias(1, bias1, BETA1, POFF1)
                emit_cd_tile(0, 1)
                emit_cd_tile(0, 2)
                post_bias(2, bias2, BETA2, POFF2)
                emit_cd_tile(0, 3)
                emit_cycle_head(0, 0)
                emit_cd_tile(0, 4)
                emit_cycle_head(0, 1)
                emit_cd_tile(0, 5)
                emit_cycle_head(1, 0)
                emit_cd_tile(0, 6)
                emit_cycle_head(1, 1)
                emit_cd_tile(0, 7)
                # (side, nst, mg chunk) trickled through the main loop
                cjobs = {9: (0, 0, (0, 1)), 13: (0, 0, (2, 3)),
                         17: (0, 1, (0, 1)), 21: (0, 1, (2, 3)),
                         25: (1, 0, (0, 1)), 29: (1, 1, (0, 1))}
                csums = {15: (0, 0), 23: (0, 1), 31: (1, 0), 35: (1, 1)}
                for jt in range(8, 64):
                    # DVE-lane member of the pair first
                    if LANE1[jt] or not LANE2[jt - 8]:
                        emit_cd_tile(0, jt)
                        emit_cd_tile(1, jt - 8)
                    else:
                        emit_cd_tile(1, jt - 8)
                        emit_cd_tile(0, jt)
                    if jt in cjobs:
                        emit_cycle_mgs(*cjobs[jt])
                    if jt in csums:
                        emit_cycle_sum(*csums[jt])
                    if jt == 55:
                        nc.sync.dma_start(out=o_min2[:, 0:48],
                                          in_=o2_sb[:, 0:48])
                nc.sync.dma_start(out=o_min1[:], in_=o1_sb)
                for jt in range(56, 64):
                    emit_cd_tile(1, jt)
                psp.release()
                nc.sync.dma_start(out=o_min2[:, 48:64], in_=o2_sb[:, 48:64])

    _legalize_sync(nc)
    nc.finalize()
    return nc


def _host_prep(inputs):
    """Gather/transpose/cast on host -> per-core input maps + bb norms."""
    xw = np.asarray(inputs['x_weight'], dtype=np.float32)
    yw = np.asarray(inputs['y_weight'], dtype=np.float32)
    xp = np.asarray(inputs['x_present']).astype(np.int64)
    yc = np.asarray(inputs['y_check']).astype(np.int64)
    yp = np.asarray(inputs['y_present']).astype(np.int64)
    xc = np.asarray(inputs['x_check']).astype(np.int64)

    def c(a, dt):
        return np.ascontiguousarray(a, dtype=dt)

    # packed bf16 const blob
    blobB = np.zeros((P, NBLOB), dtype=BF)
    fxW1 = np.asarray(inputs['fx_W1'], dtype=np.float32)  # [DX, H]
    gyW1 = np.asarray(inputs['gy_W1'], dtype=np.float32)  # [DY, H]
    for g in range(GX):
        blobB[:, CW_FX1 + g * H:CW_FX1 + (g + 1) * H] = \
            fxW1[g * P:(g + 1) * P, :].astype(BF)
    for g in range(GY):
        blobB[:, CW_GY1 + g * H:CW_GY1 + (g + 1) * H] = \
            gyW1[g * P:(g + 1) * P, :].astype(BF)
    blobB[0:H, CW_FX2:CW_FX2 + DY] = np.asarray(inputs['fx_W2']).astype(BF)
    blobB[0:H, CW_GY2:CW_GY2 + DX] = np.asarray(inputs['gy_W2']).astype(BF)
    blobB[:, C_ONES:C_ONES + P] = np.ones((P, P), dtype=BF)
    blobB[:, C_NEGI:C_NEGI + P] = (-np.eye(P)).astype(BF)
    blobB[0:1, C_ONESR:C_ONESR + 512] = np.ones((1, 512), dtype=BF)
    blobB[0:1, C_FXB2T:C_FXB2T + DY] = \
        np.asarray(inputs['fx_b2']).reshape(1, -1).astype(BF)
    blobB[0:1, C_GYB2T:C_GYB2T + DX] = \
        np.asarray(inputs['gy_b2']).reshape(1, -1).astype(BF)

    blobF = np.zeros((P, 8), dtype=F32)
    blobF[0:H, 0] = np.asarray(inputs['fx_b1']).reshape(-1)
    blobF[0:H, 1] = np.asarray(inputs['gy_b1']).reshape(-1)
    blobF[:, 2:4] = np.asarray(inputs['fx_b2']).reshape(MY, P).T
    blobF[:, 4:8] = np.asarray(inputs['gy_b2']).reshape(MX, P).T

    ycr = yw[yc]
    xcr = xw[xc]
    shared = {
        'blobB': blobB,
        'blobF': blobF,
        'ones2in': np.ones((1, 2, P), dtype=F8),
        'ycT2': c(-2.0 * ycr.T, F8),
        'xcT2': c(-2.0 * xcr.T, F8),
    }
    in_maps = []
    for cix in range(8):
        sl = slice(cix * S, (cix + 1) * S)
        m = dict(shared)
        m['xpTsb'] = c(xw[xp[sl]].T, BF)
        m['ypTsb'] = c(yw[yp[sl]].T, BF)
        in_maps.append(m)
    bb1 = np.sum(ycr.astype(np.float64) ** 2, axis=1)
    bb2 = np.sum(xcr.astype(np.float64) ** 2, axis=1)
    return in_maps, bb1, bb2


def _combine_side(results, key, ckey, lanes, beta, poff, bb):
    """Column-combine one cdist: exact-min cols by min over shards, softmin
    cols by pivot-rescaled sumexp; then +bb, clamp, sqrt, sum."""
    # device broadcasts the pivot through a bf16 1-element matmul;
    # replicate that rounding exactly
    pivots = [float(np.float32(r[ckey][0, 0]).astype(BF)) - poff
              for r in results]
    cstar = min(pivots)
    stot = np.zeros((P, 64), np.float64)
    for r, pv in zip(results, pivots):
        stot += r[key].astype(np.float64) * np.exp(beta * (cstar - pv))
    soft = cstar - np.log(np.maximum(stot, 1e-300)) / beta
    hard = np.min(np.stack([r[key] for r in results]),
                  axis=0).astype(np.float64)
    comb = np.where(np.asarray(lanes)[None, :], hard, soft)
    d = comb.T.reshape(-1) + bb
    return np.sqrt(np.maximum(d, 0.0)).sum()


def _host_combine(results, bb1, bb2):
    tot = _combine_side(results, 'o_min1', 'o_c1', LANE1, BETA1, POFF1, bb1)
    tot += _combine_side(results, 'o_min2', 'o_c2', LANE2, BETA2, POFF2, bb2)
    for r in results:
        tot += np.sqrt(np.maximum(
            r['o_cfx'].astype(np.float64).reshape(-1), 0.0)).sum()
        tot += np.sqrt(np.maximum(
            r['o_cgy'].astype(np.float64).reshape(-1), 0.0)).sum()
    return np.array(tot / float(B), dtype=np.float32)


def kernel(**inputs):
    from concourse.bass_utils import run_bass_kernel_spmd

    if 'nc' not in _CACHE:
        _CACHE['nc'] = _build_nc()
    nc = _CACHE['nc']
    in_maps, bb1, bb2 = _host_prep(inputs)
    res = run_bass_kernel_spmd(nc, in_maps, core_ids=list(range(8)),
                               trace=TRACE)
    if TRACE and res.exec_time_ns is not None:
        print(f"HW exec time: {res.exec_time_ns} ns")
        _CACHE['last_exec_ns'] = res.exec_time_ns
        _CACHE['last_trace'] = res.instructions_and_trace
    return _host_combine(res.results, bb1, bb2)
